# revision 5
# baseline (speedup 1.0000x reference)
"""DSFusion kernel for 8x TRN2 NeuronCores.

Computation (per reference):
    out_x = x @ Wx.T + bx ; out_y = y @ Wy.T + by
    sp1 = softplus(out_x) ; sp2 = softplus(out_y)
    alpha_x = sp1 + 1 ; alpha_y = sp2 + 1
    alpha_a = sp1*sp2/C + sp1 + sp2 + 1        (algebraic collapse of the
                                                Dempster-Shafer combination --
                                                all S/b/u/conflict terms cancel)

Sharding: data-parallel over the batch dim, 1024 rows per core; weights and
biases replicated. Host pre-transposes x/y/W to put the contraction dim on
partitions and pre-casts matmul operands to bf16 (fp32 PSUM accumulate).
"""

import os
import numpy as np
import ml_dtypes

BATCH = 8192
DIM = 2048
CLASSES = 1000
NCORES = 8
R = BATCH // NCORES          # rows per core (1024)
P = 128
KCH = DIM // P               # contraction chunks (16)
RT = R // P                  # row tiles per core (8)
NH = CLASSES // 2            # psum half (500, fits one 2KB bank)

_CACHE = {}

# Results of the last device run (for the test harness to inspect timing).
LAST_RESULTS = None


def _split_waits(nc, limit=1):
    """The installed walrus can't lower an instruction carrying more than one
    sync wait. Hoist extra waits onto single-wait NOPs inserted immediately
    before the instruction on the same engine (program order preserves the
    wait-all semantics)."""
    import concourse.mybir as mybir

    for f in nc.m.functions:
        for bb in f.blocks:
            out = []
            changed = False
            for ins in list(bb.instructions):
                si = ins.sync_info
                if si is not None and len(si.on_wait) > limit:
                    waits = list(si.on_wait)
                    extra, keep = waits[:-limit], waits[-limit:]
                    for i, w in enumerate(extra):
                        nop = mybir.InstNoOp(name=f"{ins.name}-ws{i}", ins=[], outs=[])
                        nop.engine = ins.engine
                        nop.sync_info = mybir.SyncInfo(on_wait=[w], on_update=[])
                        out.append(nop)
                    ins.sync_info = mybir.SyncInfo(
                        on_wait=keep, on_update=list(si.on_update)
                    )
                    changed = True
                out.append(ins)
            if changed:
                bb.instructions = out


def _build_nc():
    import concourse.bass as bass
    import concourse.mybir as mybir
    import concourse.tile as tile

    dt = mybir.dt

    nc = bass.Bass()

    xT = nc.dram_tensor("xT", [DIM, R], dt.bfloat16, kind="ExternalInput")
    yT = nc.dram_tensor("yT", [DIM, R], dt.bfloat16, kind="ExternalInput")
    wxT = nc.dram_tensor("wxT", [DIM, CLASSES], dt.bfloat16, kind="ExternalInput")
    wyT = nc.dram_tensor("wyT", [DIM, CLASSES], dt.bfloat16, kind="ExternalInput")
    bxb = nc.dram_tensor("bxb", [P, CLASSES], dt.float32, kind="ExternalInput")
    byb = nc.dram_tensor("byb", [P, CLASSES], dt.float32, kind="ExternalInput")

    aa_d = nc.dram_tensor("alpha_a", [R, CLASSES], dt.float32, kind="ExternalOutput")
    ax_d = nc.dram_tensor("alpha_x", [R, CLASSES], dt.float32, kind="ExternalOutput")
    ay_d = nc.dram_tensor("alpha_y", [R, CLASSES], dt.float32, kind="ExternalOutput")

    xT3 = xT.rearrange("(ko p) r -> p ko r", p=P)
    yT3 = yT.rearrange("(ko p) r -> p ko r", p=P)
    wxT3 = wxT.rearrange("(ko p) c -> p ko c", p=P)
    wyT3 = wyT.rearrange("(ko p) c -> p ko c", p=P)
    aa3 = aa_d.rearrange("(t p) c -> t p c", p=P)
    ax3 = ax_d.rearrange("(t p) c -> t p c", p=P)
    ay3 = ay_d.rearrange("(t p) c -> t p c", p=P)

    # softplus(x) = ln(exp(x) + 1); the installed ACT tables have no direct
    # softplus, but exp and ln share one table set. Pre-activation values are
    # within +-4 so exp cannot overflow.
    EXP = mybir.ActivationFunctionType.Exp
    LN = mybir.ActivationFunctionType.Ln
    ADD = mybir.AluOpType.add
    MULT = mybir.AluOpType.mult

    with tile.TileContext(nc) as tc:
        with (
            tc.tile_pool(name="wpool", bufs=1) as wpool,
            tc.tile_pool(name="xpool", bufs=1) as xpool,
            tc.tile_pool(name="epool", bufs=2) as epool,
            tc.tile_pool(name="psum", bufs=2, space="PSUM") as ppool,
        ):
            bx_sb = wpool.tile([P, CLASSES], dt.float32, tag="bx")
            nc.sync.dma_start(bx_sb[:], bxb[:])
            by_sb = wpool.tile([P, CLASSES], dt.float32, tag="by")
            nc.sync.dma_start(by_sb[:], byb[:])

            wx_sb, wy_sb, x_sb, y_sb = [], [], [], []
            for k in range(KCH):
                wk = wpool.tile([P, CLASSES], dt.bfloat16, tag=f"wx{k}")
                nc.sync.dma_start(wk[:], wxT3[:, k, :])
                wx_sb.append(wk)
                xk = xpool.tile([P, R], dt.bfloat16, tag=f"x{k}")
                nc.sync.dma_start(xk[:], xT3[:, k, :])
                x_sb.append(xk)
                wk = wpool.tile([P, CLASSES], dt.bfloat16, tag=f"wy{k}")
                nc.sync.dma_start(wk[:], wyT3[:, k, :])
                wy_sb.append(wk)
                yk = xpool.tile([P, R], dt.bfloat16, tag=f"y{k}")
                nc.sync.dma_start(yk[:], yT3[:, k, :])
                y_sb.append(yk)

            for r in range(RT):
                rs = slice(r * P, (r + 1) * P)
                psx0 = ppool.tile([P, NH], dt.float32, tag="psx0")
                psx1 = ppool.tile([P, NH], dt.float32, tag="psx1")
                psy0 = ppool.tile([P, NH], dt.float32, tag="psy0")
                psy1 = ppool.tile([P, NH], dt.float32, tag="psy1")
                for k in range(KCH):
                    st = k == 0
                    sp = k == KCH - 1
                    nc.tensor.matmul(psx0[:], x_sb[k][:, rs], wx_sb[k][:, 0:NH], start=st, stop=sp)
                    nc.tensor.matmul(psx1[:], x_sb[k][:, rs], wx_sb[k][:, NH:CLASSES], start=st, stop=sp)
                    nc.tensor.matmul(psy0[:], y_sb[k][:, rs], wy_sb[k][:, 0:NH], start=st, stop=sp)
                    nc.tensor.matmul(psy1[:], y_sb[k][:, rs], wy_sb[k][:, NH:CLASSES], start=st, stop=sp)

                # epilogue: bias add (free-dim bias -> DVE tensor_tensor),
                # softplus on ACT, fusion arithmetic on DVE
                t1 = epool.tile([P, CLASSES], dt.float32, tag="t1")
                nc.vector.tensor_tensor(t1[:, 0:NH], psx0[:], bx_sb[:, 0:NH], ADD)
                nc.vector.tensor_tensor(t1[:, NH:CLASSES], psx1[:], bx_sb[:, NH:CLASSES], ADD)
                t2 = epool.tile([P, CLASSES], dt.float32, tag="t2")
                nc.vector.tensor_tensor(t2[:, 0:NH], psy0[:], by_sb[:, 0:NH], ADD)
                nc.vector.tensor_tensor(t2[:, NH:CLASSES], psy1[:], by_sb[:, NH:CLASSES], ADD)

                sp1 = epool.tile([P, CLASSES], dt.float32, tag="sp1")
                nc.scalar.activation(sp1[:], t1[:], EXP)
                nc.scalar.activation(sp1[:], sp1[:], LN, bias=1.0)
                sp2 = epool.tile([P, CLASSES], dt.float32, tag="sp2")
                nc.scalar.activation(sp2[:], t2[:], EXP)
                nc.scalar.activation(sp2[:], sp2[:], LN, bias=1.0)

                ax = epool.tile([P, CLASSES], dt.float32, tag="ax")
                nc.vector.tensor_scalar_add(ax[:], sp1[:], 1.0)
                ay = epool.tile([P, CLASSES], dt.float32, tag="ay")
                nc.vector.tensor_scalar_add(ay[:], sp2[:], 1.0)
                w = epool.tile([P, CLASSES], dt.float32, tag="w")
                nc.vector.tensor_scalar(w[:], sp2[:], 1.0 / CLASSES, 1.0, MULT, ADD)
                aa = epool.tile([P, CLASSES], dt.float32, tag="aa")
                nc.vector.tensor_tensor(aa[:], sp1[:], w[:], MULT)
                nc.vector.tensor_tensor(aa[:], aa[:], ay[:], ADD)

                nc.sync.dma_start(ax3[r], ax[:])
                nc.sync.dma_start(ay3[r], ay[:])
                nc.sync.dma_start(aa3[r], aa[:])

    _split_waits(nc)
    return nc


def kernel(x, y, Wx, bx, Wy, by):
    global LAST_RESULTS
    from concourse.bass_utils import run_bass_kernel_spmd

    if "nc" not in _CACHE:
        _CACHE["nc"] = _build_nc()
    nc = _CACHE["nc"]

    bf16 = ml_dtypes.bfloat16
    x = np.asarray(x, dtype=np.float32)
    y = np.asarray(y, dtype=np.float32)
    xTb = np.ascontiguousarray(np.asarray(x).astype(bf16).T)      # [DIM, BATCH]
    yTb = np.ascontiguousarray(np.asarray(y).astype(bf16).T)
    wxT = np.ascontiguousarray(np.asarray(Wx, dtype=np.float32).astype(bf16).T)  # [DIM, CLASSES]
    wyT = np.ascontiguousarray(np.asarray(Wy, dtype=np.float32).astype(bf16).T)
    bxb = np.ascontiguousarray(np.broadcast_to(np.asarray(bx, dtype=np.float32), (P, CLASSES)))
    byb = np.ascontiguousarray(np.broadcast_to(np.asarray(by, dtype=np.float32), (P, CLASSES)))

    in_maps = []
    for c in range(NCORES):
        rs = slice(c * R, (c + 1) * R)
        in_maps.append(
            {
                "xT": np.ascontiguousarray(xTb[:, rs]),
                "yT": np.ascontiguousarray(yTb[:, rs]),
                "wxT": wxT,
                "wyT": wyT,
                "bxb": bxb,
                "byb": byb,
            }
        )

    res = run_bass_kernel_spmd(nc, in_maps, core_ids=list(range(NCORES)))
    LAST_RESULTS = res

    aa = np.concatenate([res.results[c]["alpha_a"] for c in range(NCORES)], axis=0)
    ax = np.concatenate([res.results[c]["alpha_x"] for c in range(NCORES)], axis=0)
    ay = np.concatenate([res.results[c]["alpha_y"] for c in range(NCORES)], axis=0)
    return (aa, ax, ay)


# revision 12
# speedup vs baseline: 1.0332x; 1.0332x over previous
"""DSFusion kernel for 8x TRN2 NeuronCores.

Computation (per reference):
    out_x = x @ Wx.T + bx ; out_y = y @ Wy.T + by
    sp1 = softplus(out_x) ; sp2 = softplus(out_y)
    alpha_x = sp1 + 1 ; alpha_y = sp2 + 1
    alpha_a = sp1*sp2/C + sp1 + sp2 + 1        (algebraic collapse of the
                                                Dempster-Shafer combination --
                                                all S/b/u/conflict terms cancel)

Sharding: data-parallel over the batch dim, 1024 rows per core; weights and
biases replicated. Host pre-transposes x/y/W so the contraction dim sits on
SBUF partitions and pre-casts matmul operands to bf16 (fp32 PSUM accumulate).

Schedule: rows are processed in 4 pairs of 128-row tiles. Per pair, an
X-phase accumulates out_x for both row tiles over all 16 K-chunks (4 PSUM
banks), spills psum+bias to SBUF, then a Y-phase does the same for out_y.
x/y are laid out host-side as contiguous [K-chunk, pair] tiles and DMAs are
issued in first-touch order, so the PE starts within ~2us and never waits
on the bulk weight load.
"""

import numpy as np
import ml_dtypes

BATCH = 8192
DIM = 2048
CLASSES = 1000
NCORES = 8
R = BATCH // NCORES          # rows per core (1024)
P = 128
KCH = DIM // P               # contraction chunks (16)
NPAIR = 4                    # pairs of 128-row tiles per core
PW = 2 * P                   # rows per pair (256)
NH = CLASSES // 2            # psum half (500, fits one 2KB bank)

_CACHE = {}

# Results of the last device run (for the test harness to inspect timing).
LAST_RESULTS = None


def _split_waits(nc, limit=1):
    """The installed walrus can't lower an instruction carrying more than one
    sync wait. Hoist extra waits onto single-wait NOPs inserted immediately
    before the instruction on the same engine (program order preserves the
    wait-all semantics)."""
    import concourse.mybir as mybir

    for f in nc.m.functions:
        for bb in f.blocks:
            out = []
            changed = False
            for ins in list(bb.instructions):
                si = ins.sync_info
                if si is not None and len(si.on_wait) > limit:
                    waits = list(si.on_wait)
                    extra, keep = waits[:-limit], waits[-limit:]
                    for i, w in enumerate(extra):
                        nop = mybir.InstNoOp(name=f"{ins.name}-ws{i}", ins=[], outs=[])
                        nop.engine = ins.engine
                        nop.sync_info = mybir.SyncInfo(on_wait=[w], on_update=[])
                        out.append(nop)
                    ins.sync_info = mybir.SyncInfo(
                        on_wait=keep, on_update=list(si.on_update)
                    )
                    changed = True
                out.append(ins)
            if changed:
                bb.instructions = out


def _build_nc():
    import concourse.bass as bass
    import concourse.mybir as mybir
    import concourse.tile as tile

    dt = mybir.dt

    nc = bass.Bass()

    xT = nc.dram_tensor("xT", [DIM, R], dt.bfloat16, kind="ExternalInput")
    yT = nc.dram_tensor("yT", [DIM, R], dt.bfloat16, kind="ExternalInput")
    wxT = nc.dram_tensor("wxT", [DIM, CLASSES], dt.bfloat16, kind="ExternalInput")
    wyT = nc.dram_tensor("wyT", [DIM, CLASSES], dt.bfloat16, kind="ExternalInput")
    bxb = nc.dram_tensor("bxb", [P, CLASSES], dt.bfloat16, kind="ExternalInput")
    byb = nc.dram_tensor("byb", [P, CLASSES], dt.bfloat16, kind="ExternalInput")

    aa_d = nc.dram_tensor("alpha_a", [R, CLASSES], dt.float32, kind="ExternalOutput")
    ax_d = nc.dram_tensor("alpha_x", [R, CLASSES], dt.float32, kind="ExternalOutput")
    ay_d = nc.dram_tensor("alpha_y", [R, CLASSES], dt.float32, kind="ExternalOutput")

    xT3 = xT.rearrange("(ko p) r -> p ko r", p=P)
    yT3 = yT.rearrange("(ko p) r -> p ko r", p=P)
    wxT3 = wxT.rearrange("(ko p) c -> p ko c", p=P)
    wyT3 = wyT.rearrange("(ko p) c -> p ko c", p=P)
    aa3 = aa_d.rearrange("(t p) c -> t p c", p=P)
    ax3 = ax_d.rearrange("(t p) c -> t p c", p=P)
    ay3 = ay_d.rearrange("(t p) c -> t p c", p=P)

    # softplus(x) = ln(exp(x) + 1); the installed ACT tables have no direct
    # softplus, but exp and ln share one table set. Pre-activation values are
    # within +-4 so exp cannot overflow.
    EXP = mybir.ActivationFunctionType.Exp
    LN = mybir.ActivationFunctionType.Ln
    ADD = mybir.AluOpType.add
    MULT = mybir.AluOpType.mult

    with tile.TileContext(nc) as tc:
        with (
            tc.tile_pool(name="wpool", bufs=1) as wpool,
            tc.tile_pool(name="xpool", bufs=1) as xpool,
            tc.tile_pool(name="epool", bufs=1) as epool,
            tc.tile_pool(name="opool", bufs=2) as opool,
            tc.tile_pool(name="psum", bufs=1, space="PSUM") as ppool,
        ):
            # -- input DMAs in first-touch order --------------------------
            # DMA triggers cost ~0.6us of *serial* sequencer time each, so
            # x/wx issue from the SP sequencer and y/wy from the ACT
            # sequencer (both are HWDGE), and the pair-1..3 bulk is batched
            # 2 K-chunks per DMA.
            x0_sb, y0_sb, wx_sb, wy_sb = [], [], [], []

            for k in range(KCH):  # pair-0 slices + weights, one K-chunk per step
                t_ = xpool.tile([P, PW], dt.bfloat16, tag=f"x{k}_0")
                nc.sync.dma_start(t_[:], xT3[:, k, 0:PW])
                x0_sb.append(t_)
                t_ = xpool.tile([P, PW], dt.bfloat16, tag=f"y{k}_0")
                nc.scalar.dma_start(t_[:], yT3[:, k, 0:PW])
                y0_sb.append(t_)
                t_ = wpool.tile([P, CLASSES], dt.bfloat16, tag=f"wx{k}")
                nc.sync.dma_start(t_[:], wxT3[:, k, :])
                wx_sb.append(t_)
                t_ = wpool.tile([P, CLASSES], dt.bfloat16, tag=f"wy{k}")
                nc.scalar.dma_start(t_[:], wyT3[:, k, :])
                wy_sb.append(t_)

            bx_sb = wpool.tile([P, CLASSES], dt.bfloat16, tag="bx")
            nc.sync.dma_start(bx_sb[:], bxb[:])
            by_sb = wpool.tile([P, CLASSES], dt.bfloat16, tag="by")
            nc.scalar.dma_start(by_sb[:], byb[:])

            # pairs 1..3: [2 K-chunks x 3 pairs x PW] per DMA (384KB each)
            x1_sb, y1_sb = [], []
            for kk in range(KCH // 2):
                t_ = xpool.tile([P, 2, (NPAIR - 1) * PW], dt.bfloat16, tag=f"x{kk}_b")
                nc.sync.dma_start(t_[:], xT3[:, 2 * kk:2 * kk + 2, PW:R])
                x1_sb.append(t_)
                t_ = xpool.tile([P, 2, (NPAIR - 1) * PW], dt.bfloat16, tag=f"y{kk}_b")
                nc.scalar.dma_start(t_[:], yT3[:, 2 * kk:2 * kk + 2, PW:R])
                y1_sb.append(t_)

            def x_slice(k, pr, j):
                if pr == 0:
                    return x0_sb[k][:, j * P:(j + 1) * P]
                return x1_sb[k // 2][:, k % 2, (pr - 1) * PW + j * P:(pr - 1) * PW + (j + 1) * P]

            def y_slice(k, pr, j):
                if pr == 0:
                    return y0_sb[k][:, j * P:(j + 1) * P]
                return y1_sb[k // 2][:, k % 2, (pr - 1) * PW + j * P:(pr - 1) * PW + (j + 1) * P]

            # -- compute ---------------------------------------------------
            for pr in range(NPAIR):
                # X phase: out_x for both row tiles of this pair
                psx = [
                    [ppool.tile([P, NH], dt.float32, tag=f"psx{h}_{j}", name=f"psx{h}_{j}") for h in range(2)]
                    for j in range(2)
                ]
                for k in range(KCH):
                    st, sp = k == 0, k == KCH - 1
                    for j in range(2):
                        lhsT = x_slice(k, pr, j)
                        nc.tensor.matmul(psx[j][0][:], lhsT, wx_sb[k][:, 0:NH], start=st, stop=sp)
                        nc.tensor.matmul(psx[j][1][:], lhsT, wx_sb[k][:, NH:CLASSES], start=st, stop=sp)
                t1 = []
                for j in range(2):
                    t_ = epool.tile([P, CLASSES], dt.float32, tag=f"t1_{j}")
                    nc.vector.tensor_tensor(t_[:, 0:NH], psx[j][0][:], bx_sb[:, 0:NH], ADD)
                    nc.vector.tensor_tensor(t_[:, NH:CLASSES], psx[j][1][:], bx_sb[:, NH:CLASSES], ADD)
                    t1.append(t_)

                # Y phase
                psy = [
                    [ppool.tile([P, NH], dt.float32, tag=f"psy{h}_{j}", name=f"psy{h}_{j}") for h in range(2)]
                    for j in range(2)
                ]
                for k in range(KCH):
                    st, sp = k == 0, k == KCH - 1
                    for j in range(2):
                        lhsT = y_slice(k, pr, j)
                        nc.tensor.matmul(psy[j][0][:], lhsT, wy_sb[k][:, 0:NH], start=st, stop=sp)
                        nc.tensor.matmul(psy[j][1][:], lhsT, wy_sb[k][:, NH:CLASSES], start=st, stop=sp)
                t2 = []
                for j in range(2):
                    t_ = epool.tile([P, CLASSES], dt.float32, tag=f"t2_{j}")
                    nc.vector.tensor_tensor(t_[:, 0:NH], psy[j][0][:], by_sb[:, 0:NH], ADD)
                    nc.vector.tensor_tensor(t_[:, NH:CLASSES], psy[j][1][:], by_sb[:, NH:CLASSES], ADD)
                    t2.append(t_)

                # epilogue per row tile: softplus, fusion, store
                for j in range(2):
                    r = 2 * pr + j
                    sp1, sp2 = t1[j], t2[j]
                    nc.scalar.activation(sp1[:], sp1[:], EXP)
                    nc.scalar.activation(sp1[:], sp1[:], LN, bias=1.0)
                    nc.scalar.activation(sp2[:], sp2[:], EXP)
                    nc.scalar.activation(sp2[:], sp2[:], LN, bias=1.0)

                    ax = opool.tile([P, CLASSES], dt.float32, tag="ax")
                    nc.vector.tensor_scalar_add(ax[:], sp1[:], 1.0)
                    nc.sync.dma_start(ax3[r], ax[:])
                    ay = opool.tile([P, CLASSES], dt.float32, tag="ay")
                    nc.vector.tensor_scalar_add(ay[:], sp2[:], 1.0)
                    nc.scalar.dma_start(ay3[r], ay[:])
                    # aa = sp1*(sp2/C + 1) + (sp2 + 1), built in-place:
                    nc.vector.tensor_scalar(sp2[:], sp2[:], 1.0 / CLASSES, 1.0, MULT, ADD)
                    nc.vector.tensor_tensor(sp1[:], sp1[:], sp2[:], MULT)
                    aa = opool.tile([P, CLASSES], dt.float32, tag="aa")
                    nc.vector.tensor_tensor(aa[:], sp1[:], ay[:], ADD)
                    nc.sync.dma_start(aa3[r], aa[:])

    _split_waits(nc)
    return nc


def kernel(x, y, Wx, bx, Wy, by):
    global LAST_RESULTS
    from concourse.bass_utils import run_bass_kernel_spmd

    if "nc" not in _CACHE:
        _CACHE["nc"] = _build_nc()
    nc = _CACHE["nc"]

    bf16 = ml_dtypes.bfloat16
    x = np.asarray(x, dtype=np.float32)
    y = np.asarray(y, dtype=np.float32)
    xb = x.astype(bf16)                       # [BATCH, DIM]
    yb = y.astype(bf16)
    wxT = np.ascontiguousarray(np.asarray(Wx, dtype=np.float32).astype(bf16).T)  # [DIM, CLASSES]
    wyT = np.ascontiguousarray(np.asarray(Wy, dtype=np.float32).astype(bf16).T)
    bxb = np.ascontiguousarray(
        np.broadcast_to(np.asarray(bx, dtype=np.float32).astype(bf16), (P, CLASSES))
    )
    byb = np.ascontiguousarray(
        np.broadcast_to(np.asarray(by, dtype=np.float32).astype(bf16), (P, CLASSES))
    )

    xTb = np.ascontiguousarray(xb.T)          # [DIM, BATCH]
    yTb = np.ascontiguousarray(yb.T)

    in_maps = []
    for c in range(NCORES):
        rs = slice(c * R, (c + 1) * R)
        in_maps.append(
            {
                "xT": np.ascontiguousarray(xTb[:, rs]),
                "yT": np.ascontiguousarray(yTb[:, rs]),
                "wxT": wxT,
                "wyT": wyT,
                "bxb": bxb,
                "byb": byb,
            }
        )

    res = run_bass_kernel_spmd(nc, in_maps, core_ids=list(range(NCORES)))
    LAST_RESULTS = res

    aa = np.concatenate([res.results[c]["alpha_a"] for c in range(NCORES)], axis=0)
    ax = np.concatenate([res.results[c]["alpha_x"] for c in range(NCORES)], axis=0)
    ay = np.concatenate([res.results[c]["alpha_y"] for c in range(NCORES)], axis=0)
    return (aa, ax, ay)


# revision 13
# speedup vs baseline: 1.0396x; 1.0062x over previous
"""DSFusion kernel for 8x TRN2 NeuronCores.

Computation (per reference):
    out_x = x @ Wx.T + bx ; out_y = y @ Wy.T + by
    sp1 = softplus(out_x) ; sp2 = softplus(out_y)
    alpha_x = sp1 + 1 ; alpha_y = sp2 + 1
    alpha_a = sp1*sp2/C + sp1 + sp2 + 1        (algebraic collapse of the
                                                Dempster-Shafer combination --
                                                all S/b/u/conflict terms cancel)

Sharding: data-parallel over the batch dim, 1024 rows per core; weights and
biases replicated. Host pre-transposes x/y/W so the contraction dim sits on
SBUF partitions and pre-casts matmul operands to bf16 (fp32 PSUM accumulate).

Schedule: rows are processed in 4 pairs of 128-row tiles. Per pair, an
X-phase accumulates out_x for both row tiles over all 16 K-chunks (4 PSUM
banks), spills psum+bias to SBUF, then a Y-phase does the same for out_y.
x/y are laid out host-side as contiguous [K-chunk, pair] tiles and DMAs are
issued in first-touch order, so the PE starts within ~2us and never waits
on the bulk weight load.
"""

import numpy as np
import ml_dtypes

BATCH = 8192
DIM = 2048
CLASSES = 1000
NCORES = 8
R = BATCH // NCORES          # rows per core (1024)
P = 128
KCH = DIM // P               # contraction chunks (16)
NPAIR = 4                    # pairs of 128-row tiles per core
PW = 2 * P                   # rows per pair (256)
NH = CLASSES // 2            # psum half (500, fits one 2KB bank)

_CACHE = {}

# Results of the last device run (for the test harness to inspect timing).
LAST_RESULTS = None


def _split_waits(nc, limit=1):
    """The installed walrus can't lower an instruction carrying more than one
    sync wait. Hoist extra waits onto single-wait NOPs inserted immediately
    before the instruction on the same engine (program order preserves the
    wait-all semantics)."""
    import concourse.mybir as mybir

    for f in nc.m.functions:
        for bb in f.blocks:
            out = []
            changed = False
            for ins in list(bb.instructions):
                si = ins.sync_info
                if si is not None and len(si.on_wait) > limit:
                    waits = list(si.on_wait)
                    extra, keep = waits[:-limit], waits[-limit:]
                    for i, w in enumerate(extra):
                        nop = mybir.InstNoOp(name=f"{ins.name}-ws{i}", ins=[], outs=[])
                        nop.engine = ins.engine
                        nop.sync_info = mybir.SyncInfo(on_wait=[w], on_update=[])
                        out.append(nop)
                    ins.sync_info = mybir.SyncInfo(
                        on_wait=keep, on_update=list(si.on_update)
                    )
                    changed = True
                out.append(ins)
            if changed:
                bb.instructions = out


def _build_nc():
    import concourse.bass as bass
    import concourse.mybir as mybir
    import concourse.tile as tile

    dt = mybir.dt

    nc = bass.Bass()

    xT = nc.dram_tensor("xT", [DIM, R], dt.bfloat16, kind="ExternalInput")
    yT = nc.dram_tensor("yT", [DIM, R], dt.bfloat16, kind="ExternalInput")
    wxT = nc.dram_tensor("wxT", [DIM, CLASSES], dt.bfloat16, kind="ExternalInput")
    wyT = nc.dram_tensor("wyT", [DIM, CLASSES], dt.bfloat16, kind="ExternalInput")
    bxb = nc.dram_tensor("bxb", [P, CLASSES], dt.bfloat16, kind="ExternalInput")
    byb = nc.dram_tensor("byb", [P, CLASSES], dt.bfloat16, kind="ExternalInput")

    aa_d = nc.dram_tensor("alpha_a", [R, CLASSES], dt.float32, kind="ExternalOutput")
    ax_d = nc.dram_tensor("alpha_x", [R, CLASSES], dt.float32, kind="ExternalOutput")
    ay_d = nc.dram_tensor("alpha_y", [R, CLASSES], dt.float32, kind="ExternalOutput")

    xT3 = xT.rearrange("(ko p) r -> p ko r", p=P)
    yT3 = yT.rearrange("(ko p) r -> p ko r", p=P)
    wxT3 = wxT.rearrange("(ko p) c -> p ko c", p=P)
    wyT3 = wyT.rearrange("(ko p) c -> p ko c", p=P)
    aa3 = aa_d.rearrange("(t p) c -> t p c", p=P)
    ax3 = ax_d.rearrange("(t p) c -> t p c", p=P)
    ay3 = ay_d.rearrange("(t p) c -> t p c", p=P)

    # softplus(x) = ln(exp(x) + 1); the installed ACT tables have no direct
    # softplus, but exp and ln share one table set. Pre-activation values are
    # within +-4 so exp cannot overflow.
    EXP = mybir.ActivationFunctionType.Exp
    LN = mybir.ActivationFunctionType.Ln
    ADD = mybir.AluOpType.add
    MULT = mybir.AluOpType.mult

    with tile.TileContext(nc) as tc:
        with (
            tc.tile_pool(name="wpool", bufs=1) as wpool,
            tc.tile_pool(name="xpool", bufs=1) as xpool,
            tc.tile_pool(name="epool", bufs=2) as epool,
            tc.tile_pool(name="opool", bufs=2) as opool,
            tc.tile_pool(name="psum", bufs=1, space="PSUM") as ppool,
        ):
            # -- input DMAs in first-touch order --------------------------
            # DMA triggers cost ~0.6us of *serial* sequencer time each, so
            # x/wx issue from the SP sequencer and y/wy from the ACT
            # sequencer (both are HWDGE), and the pair-1..3 bulk is batched
            # 2 K-chunks per DMA.
            x0_sb, y0_sb, wx_sb, wy_sb = [], [], [], []

            for k in range(KCH):  # pair-0 slices + weights, one K-chunk per step
                t_ = xpool.tile([P, PW], dt.bfloat16, tag=f"x{k}_0")
                nc.sync.dma_start(t_[:], xT3[:, k, 0:PW])
                x0_sb.append(t_)
                t_ = xpool.tile([P, PW], dt.bfloat16, tag=f"y{k}_0")
                nc.scalar.dma_start(t_[:], yT3[:, k, 0:PW])
                y0_sb.append(t_)
                t_ = wpool.tile([P, CLASSES], dt.bfloat16, tag=f"wx{k}")
                nc.sync.dma_start(t_[:], wxT3[:, k, :])
                wx_sb.append(t_)
                t_ = wpool.tile([P, CLASSES], dt.bfloat16, tag=f"wy{k}")
                nc.scalar.dma_start(t_[:], wyT3[:, k, :])
                wy_sb.append(t_)

            bx_sb = wpool.tile([P, CLASSES], dt.bfloat16, tag="bx")
            nc.sync.dma_start(bx_sb[:], bxb[:])
            by_sb = wpool.tile([P, CLASSES], dt.bfloat16, tag="by")
            nc.scalar.dma_start(by_sb[:], byb[:])

            # pairs 1..3: [2 K-chunks x 3 pairs x PW] per DMA (384KB each)
            x1_sb, y1_sb = [], []
            for kk in range(KCH // 2):
                t_ = xpool.tile([P, 2, (NPAIR - 1) * PW], dt.bfloat16, tag=f"x{kk}_b")
                nc.sync.dma_start(t_[:], xT3[:, 2 * kk:2 * kk + 2, PW:R])
                x1_sb.append(t_)
                t_ = xpool.tile([P, 2, (NPAIR - 1) * PW], dt.bfloat16, tag=f"y{kk}_b")
                nc.scalar.dma_start(t_[:], yT3[:, 2 * kk:2 * kk + 2, PW:R])
                y1_sb.append(t_)

            def x_slice(k, pr, j):
                if pr == 0:
                    return x0_sb[k][:, j * P:(j + 1) * P]
                return x1_sb[k // 2][:, k % 2, (pr - 1) * PW + j * P:(pr - 1) * PW + (j + 1) * P]

            def y_slice(k, pr, j):
                if pr == 0:
                    return y0_sb[k][:, j * P:(j + 1) * P]
                return y1_sb[k // 2][:, k % 2, (pr - 1) * PW + j * P:(pr - 1) * PW + (j + 1) * P]

            # -- compute ---------------------------------------------------
            for pr in range(NPAIR):
                # X phase: out_x for both row tiles of this pair
                psx = [
                    [ppool.tile([P, NH], dt.float32, tag=f"psx{h}_{j}", name=f"psx{h}_{j}") for h in range(2)]
                    for j in range(2)
                ]
                for k in range(KCH):
                    st, sp = k == 0, k == KCH - 1
                    for j in range(2):
                        lhsT = x_slice(k, pr, j)
                        nc.tensor.matmul(psx[j][0][:], lhsT, wx_sb[k][:, 0:NH], start=st, stop=sp)
                        nc.tensor.matmul(psx[j][1][:], lhsT, wx_sb[k][:, NH:CLASSES], start=st, stop=sp)
                t1 = []
                for j in range(2):
                    t_ = epool.tile([P, CLASSES], dt.float32, tag=f"t1_{j}")
                    nc.vector.tensor_tensor(t_[:, 0:NH], psx[j][0][:], bx_sb[:, 0:NH], ADD)
                    nc.vector.tensor_tensor(t_[:, NH:CLASSES], psx[j][1][:], bx_sb[:, NH:CLASSES], ADD)
                    t1.append(t_)

                # Y phase
                psy = [
                    [ppool.tile([P, NH], dt.float32, tag=f"psy{h}_{j}", name=f"psy{h}_{j}") for h in range(2)]
                    for j in range(2)
                ]
                for k in range(KCH):
                    st, sp = k == 0, k == KCH - 1
                    for j in range(2):
                        lhsT = y_slice(k, pr, j)
                        nc.tensor.matmul(psy[j][0][:], lhsT, wy_sb[k][:, 0:NH], start=st, stop=sp)
                        nc.tensor.matmul(psy[j][1][:], lhsT, wy_sb[k][:, NH:CLASSES], start=st, stop=sp)
                t2 = []
                for j in range(2):
                    t_ = epool.tile([P, CLASSES], dt.float32, tag=f"t2_{j}")
                    nc.vector.tensor_tensor(t_[:, 0:NH], psy[j][0][:], by_sb[:, 0:NH], ADD)
                    nc.vector.tensor_tensor(t_[:, NH:CLASSES], psy[j][1][:], by_sb[:, NH:CLASSES], ADD)
                    t2.append(t_)

                # epilogue per row tile: softplus, fusion, store
                for j in range(2):
                    r = 2 * pr + j
                    sp1, sp2 = t1[j], t2[j]
                    nc.scalar.activation(sp1[:], sp1[:], EXP)
                    nc.scalar.activation(sp1[:], sp1[:], LN, bias=1.0)
                    nc.scalar.activation(sp2[:], sp2[:], EXP)
                    nc.scalar.activation(sp2[:], sp2[:], LN, bias=1.0)

                    ax = opool.tile([P, CLASSES], dt.float32, tag="ax")
                    nc.vector.tensor_scalar_add(ax[:], sp1[:], 1.0)
                    nc.sync.dma_start(ax3[r], ax[:])
                    ay = opool.tile([P, CLASSES], dt.float32, tag="ay")
                    nc.vector.tensor_scalar_add(ay[:], sp2[:], 1.0)
                    nc.sync.dma_start(ay3[r], ay[:])
                    # aa = sp1*(sp2/C + 1) + (sp2 + 1), built in-place:
                    nc.vector.tensor_scalar(sp2[:], sp2[:], 1.0 / CLASSES, 1.0, MULT, ADD)
                    nc.vector.tensor_tensor(sp1[:], sp1[:], sp2[:], MULT)
                    aa = opool.tile([P, CLASSES], dt.float32, tag="aa")
                    nc.vector.tensor_tensor(aa[:], sp1[:], ay[:], ADD)
                    nc.sync.dma_start(aa3[r], aa[:])

    _split_waits(nc)
    return nc


def kernel(x, y, Wx, bx, Wy, by):
    global LAST_RESULTS
    from concourse.bass_utils import run_bass_kernel_spmd

    if "nc" not in _CACHE:
        _CACHE["nc"] = _build_nc()
    nc = _CACHE["nc"]

    bf16 = ml_dtypes.bfloat16
    x = np.asarray(x, dtype=np.float32)
    y = np.asarray(y, dtype=np.float32)
    xb = x.astype(bf16)                       # [BATCH, DIM]
    yb = y.astype(bf16)
    wxT = np.ascontiguousarray(np.asarray(Wx, dtype=np.float32).astype(bf16).T)  # [DIM, CLASSES]
    wyT = np.ascontiguousarray(np.asarray(Wy, dtype=np.float32).astype(bf16).T)
    bxb = np.ascontiguousarray(
        np.broadcast_to(np.asarray(bx, dtype=np.float32).astype(bf16), (P, CLASSES))
    )
    byb = np.ascontiguousarray(
        np.broadcast_to(np.asarray(by, dtype=np.float32).astype(bf16), (P, CLASSES))
    )

    xTb = np.ascontiguousarray(xb.T)          # [DIM, BATCH]
    yTb = np.ascontiguousarray(yb.T)

    in_maps = []
    for c in range(NCORES):
        rs = slice(c * R, (c + 1) * R)
        in_maps.append(
            {
                "xT": np.ascontiguousarray(xTb[:, rs]),
                "yT": np.ascontiguousarray(yTb[:, rs]),
                "wxT": wxT,
                "wyT": wyT,
                "bxb": bxb,
                "byb": byb,
            }
        )

    res = run_bass_kernel_spmd(nc, in_maps, core_ids=list(range(NCORES)))
    LAST_RESULTS = res

    aa = np.concatenate([res.results[c]["alpha_a"] for c in range(NCORES)], axis=0)
    ax = np.concatenate([res.results[c]["alpha_x"] for c in range(NCORES)], axis=0)
    ay = np.concatenate([res.results[c]["alpha_y"] for c in range(NCORES)], axis=0)
    return (aa, ax, ay)
